# revision 5
# baseline (speedup 1.0000x reference)
"""GroupQuantLinear: y = x @ dequant(w).T + b on 8 NeuronCores.

Strategy (column-parallel / tensor-parallel over out_features):
  - Host: dequantize packed 4-bit weights -> W (out,in) fp32, cast to fp16,
    pre-transpose to WT (in,out); pre-transpose x -> xT (in,tokens) fp16.
  - Shard WT / bias along out_features across 8 cores (1376 each).
  - Each core: WT shard resident in SBUF (fp16, 11.3MB); stream 128-token
    tiles of xT; fp16 matmuls accumulate over K=4096 in fp32 PSUM
    (3 PSUM banks: 512/512/352 out-cols per token tile); add bias on
    copy-out; DMA fp32 output in natural (tokens, outs) layout.
  - W is loaded in ks-major slabs so the PE can start after ~2MB arrives.
  - Host: concatenate the 8 output shards along the out dim.
"""

import os
import sys
from contextlib import ExitStack

import numpy as np

sys.path.insert(0, "/opt/trn_rl_repo")

TOKENS = 8192
IN_F = 4096
OUT_F = 11008
N_CORES = 8
SHARD = OUT_F // N_CORES          # 1376
CHUNKS = (512, 512, 352)          # out-cols per PSUM bank, sum = SHARD
P = 128
KS = IN_F // P                    # 32
TT = TOKENS // P                  # 64
W_SLAB = 1                        # ks per W-load DMA slab (after the first 4)

_NC_CACHE = {}


def _build_nc():
    import concourse.bacc as bacc
    import concourse.mybir as mybir
    import concourse.tile as tile

    dt16 = mybir.dt.float16

    nc = bacc.Bacc(
        "TRN2",
        target_bir_lowering=False,
        debug=False,
        enable_asserts=False,
        num_devices=N_CORES,
    )
    xt = nc.dram_tensor("xt", (IN_F, TOKENS), dt16, kind="ExternalInput").ap()
    wt = nc.dram_tensor("wt", (IN_F, SHARD), dt16, kind="ExternalInput").ap()
    brep = nc.dram_tensor("brep", (P, SHARD), mybir.dt.float32, kind="ExternalInput").ap()
    y = nc.dram_tensor("y", (TOKENS, SHARD), mybir.dt.float32, kind="ExternalOutput").ap()

    coff = [0]
    for ch in CHUNKS:
        coff.append(coff[-1] + ch)

    with tile.TileContext(nc) as tc, ExitStack() as ctx:
        wpool = ctx.enter_context(tc.tile_pool(name="w", bufs=1))
        xpool = ctx.enter_context(tc.tile_pool(name="x", bufs=4))
        opool = ctx.enter_context(tc.tile_pool(name="o", bufs=6))
        pspool = ctx.enter_context(tc.tile_pool(name="ps", bufs=2, space="PSUM"))

        w_sb = wpool.tile([P, KS, SHARD], dt16, name="w_sb")
        bias_sb = wpool.tile([P, SHARD], mybir.dt.float32, name="bias_sb")

        xt_r = xt.rearrange("(ks p) m -> p ks m", p=P)
        wt_r = wt.rearrange("(ks p) n -> p ks n", p=P)

        # First x tile before the weight slabs so the PE can start ASAP;
        # split across 4 DMA queues.
        x0 = xpool.tile([P, KS, P], dt16, name="x_sb", tag="x_sb")
        for q in range(4):
            nc.sync.dma_start(
                x0[:, q * (KS // 4):(q + 1) * (KS // 4), :],
                xt_r[:, q * (KS // 4):(q + 1) * (KS // 4), 0:P],
            )
        # W in ks-major slabs: PE consumes ks-sequentially during t=0.
        # First slabs split in two for earlier arrival.
        for s in range(0, 4):
            half = SHARD // 2
            nc.sync.dma_start(w_sb[:, s:s + 1, :half], wt_r[:, s:s + 1, :half])
            nc.sync.dma_start(w_sb[:, s:s + 1, half:], wt_r[:, s:s + 1, half:])
        for s in range(4, KS, W_SLAB):
            nc.sync.dma_start(
                w_sb[:, s:s + W_SLAB, :], wt_r[:, s:s + W_SLAB, :]
            )
        nc.sync.dma_start(bias_sb[:], brep)

        for t in range(TT):
            if t == 0:
                x_sb = x0
            else:
                x_sb = xpool.tile([P, KS, P], dt16, name="x_sb", tag="x_sb")
                for q in range(2):
                    nc.sync.dma_start(
                        x_sb[:, q * (KS // 2):(q + 1) * (KS // 2), :],
                        xt_r[:, q * (KS // 2):(q + 1) * (KS // 2),
                             t * P:(t + 1) * P],
                    )

            pss = [
                pspool.tile([P, CHUNKS[c]], mybir.dt.float32,
                            name=f"ps{c}", tag=f"ps{c}")
                for c in range(len(CHUNKS))
            ]
            for ks in range(KS):
                for c in range(len(CHUNKS)):
                    nc.tensor.matmul(
                        pss[c][:],
                        x_sb[:, ks, :],
                        w_sb[:, ks, coff[c]:coff[c + 1]],
                        start=(ks == 0),
                        stop=(ks == KS - 1),
                    )
            for c in range(len(CHUNKS)):
                o_sb = opool.tile([P, 512], mybir.dt.float32,
                                  name="o_sb", tag="o_sb")[:, :CHUNKS[c]]
                nc.vector.tensor_add(o_sb[:], pss[c][:], bias_sb[:, coff[c]:coff[c + 1]])
                nc.sync.dma_start(y[t * P:(t + 1) * P, coff[c]:coff[c + 1]], o_sb[:])

    nc.compile()
    return nc


def _host_prep(x, w_packed, w_scale, w_bias, b):
    import ml_dtypes  # noqa: F401

    # Dequantize on host exactly as the reference does, then cast to fp16.
    shifts = np.array([12, 8, 4, 0], dtype=np.int32)
    nib = ((w_packed[..., None] >> shifts) & 15).astype(np.float32)
    n_rows, n_groups, n_ids = w_packed.shape
    W = nib.reshape(n_rows, n_groups, n_ids * 4) * w_scale + w_bias
    W = W.reshape(n_rows, n_groups * n_ids * 4)          # (out, in) fp32
    WT = np.ascontiguousarray(W.T.astype(np.float16))    # (in, out) fp16
    xT = np.ascontiguousarray(x.T.astype(np.float16))    # (in, tokens) fp16

    in_maps = []
    for i in range(N_CORES):
        sl = slice(i * SHARD, (i + 1) * SHARD)
        in_maps.append(
            {
                "xt": xT,
                "wt": np.ascontiguousarray(WT[:, sl]),
                "brep": np.ascontiguousarray(
                    np.broadcast_to(b[sl].astype(np.float32), (P, SHARD))
                ),
            }
        )
    return in_maps


def _run(x, w_packed, w_scale, w_bias, b, trace=False):
    from concourse.bass_utils import run_bass_kernel_spmd

    if "nc" not in _NC_CACHE:
        _NC_CACHE["nc"] = _build_nc()
    nc = _NC_CACHE["nc"]
    in_maps = _host_prep(x, w_packed, w_scale, w_bias, b)
    res = run_bass_kernel_spmd(nc, in_maps, list(range(N_CORES)), trace=trace)
    y = np.concatenate([res.results[i]["y"] for i in range(N_CORES)], axis=1)
    return np.ascontiguousarray(y.astype(np.float32)), res


def kernel(x, w_packed, w_scale, w_bias, b):
    x = np.asarray(x)
    w_packed = np.asarray(w_packed)
    w_scale = np.asarray(w_scale)
    w_bias = np.asarray(w_bias)
    b = np.asarray(b)
    y, _ = _run(x, w_packed, w_scale, w_bias, b, trace=False)
    return y


# revision 9
# speedup vs baseline: 1.0986x; 1.0986x over previous
"""GroupQuantLinear: y = x @ dequant(w).T + b on 8 NeuronCores.

Strategy (column-parallel / tensor-parallel over out_features):
  - Host: dequantize packed 4-bit weights -> W (out,in) fp32, cast to fp16,
    pre-transpose to WT (in,out); pre-transpose x -> xT (in,tokens) fp16.
  - Shard WT / bias along out_features across 8 cores (1376 each).
  - Each core: WT shard resident in SBUF (fp16, 11.3MB); stream 128-token
    tiles of xT; fp16 matmuls accumulate over K=4096 in fp32 PSUM
    (3 PSUM banks: 512/512/352 out-cols per token tile); add bias on
    copy-out; DMA fp32 output in natural (tokens, outs) layout.
  - W is loaded in ks-major slabs so the PE can start after ~2MB arrives.
  - Host: concatenate the 8 output shards along the out dim.
"""

import os
import sys
from contextlib import ExitStack

import numpy as np

sys.path.insert(0, "/opt/trn_rl_repo")

TOKENS = 8192
IN_F = 4096
OUT_F = 11008
N_CORES = 8
SHARD = OUT_F // N_CORES          # 1376
CHUNKS = (512, 512, 352)          # out-cols per PSUM bank, sum = SHARD
P = 128
KS = IN_F // P                    # 32
TT = TOKENS // P                  # 64
W_SLAB = 1                        # ks per W-load DMA slab (after the first 4)

_NC_CACHE = {}


def _build_nc():
    import concourse.bacc as bacc
    import concourse.mybir as mybir
    import concourse.tile as tile

    dt16 = mybir.dt.float16

    nc = bacc.Bacc(
        "TRN2",
        target_bir_lowering=False,
        debug=False,
        enable_asserts=False,
        num_devices=N_CORES,
    )
    xt = nc.dram_tensor("xt", (IN_F, TOKENS), dt16, kind="ExternalInput").ap()
    wt = nc.dram_tensor("wt", (IN_F, SHARD), dt16, kind="ExternalInput").ap()
    brep = nc.dram_tensor("brep", (P, SHARD), mybir.dt.float32, kind="ExternalInput").ap()
    y = nc.dram_tensor("y", (TOKENS, SHARD), mybir.dt.float32, kind="ExternalOutput").ap()

    coff = [0]
    for ch in CHUNKS:
        coff.append(coff[-1] + ch)

    with tile.TileContext(nc) as tc, ExitStack() as ctx:
        wpool = ctx.enter_context(tc.tile_pool(name="w", bufs=1))
        xpool = ctx.enter_context(tc.tile_pool(name="x", bufs=4))
        opool = ctx.enter_context(tc.tile_pool(name="o", bufs=6))
        pspool = ctx.enter_context(tc.tile_pool(name="ps", bufs=2, space="PSUM"))

        w_sb = wpool.tile([P, KS, SHARD], dt16, name="w_sb")
        bias_sb = wpool.tile([P, SHARD], mybir.dt.float32, name="bias_sb")

        xt_r = xt.rearrange("(ks p) m -> p ks m", p=P)
        wt_r = wt.rearrange("(ks p) n -> p ks n", p=P)

        # PE prewarm: dependency-free dummy matmuls on uninitialized SBUF.
        # They run during the initial DMA wait and lift HAM to 2.4GHz
        # before the first real matmul issues.
        warm_in = wpool.tile([P, P], dt16, name="warm_in")
        nc.any.memzero(warm_in[:])
        warm_ps = pspool.tile([P, P], mybir.dt.float32, name="warm_ps", tag="warm", bufs=1)
        for _ in range(60):
            nc.tensor.matmul(warm_ps[:], warm_in[:], warm_in[:], start=True, stop=True)

        # Early loads, balanced so x ks-slices land just ahead of their
        # consumption by the t0/t1-interleaved ks loop.
        x0 = xpool.tile([P, KS, P], dt16, name="x_sb", tag="x_sb")
        x1 = xpool.tile([P, KS, P], dt16, name="x_sb", tag="x_sb")
        nc.sync.dma_start(x0[:, 0:4, :], xt_r[:, 0:4, 0:P])
        nc.sync.dma_start(x1[:, 0:4, :], xt_r[:, 0:4, P:2 * P])
        q4 = SHARD // 4
        for q in range(4):
            nc.sync.dma_start(
                w_sb[:, 0:1, q * q4:(q + 1) * q4], wt_r[:, 0:1, q * q4:(q + 1) * q4]
            )
        nc.sync.dma_start(x0[:, 4:16, :], xt_r[:, 4:16, 0:P])
        nc.sync.dma_start(x1[:, 4:16, :], xt_r[:, 4:16, P:2 * P])
        half = SHARD // 2
        for s in range(1, 4):
            nc.sync.dma_start(w_sb[:, s:s + 1, :half], wt_r[:, s:s + 1, :half])
            nc.sync.dma_start(w_sb[:, s:s + 1, half:], wt_r[:, s:s + 1, half:])
        nc.sync.dma_start(x0[:, 16:KS, :], xt_r[:, 16:KS, 0:P])
        nc.sync.dma_start(x1[:, 16:KS, :], xt_r[:, 16:KS, P:2 * P])
        for s in range(4, KS, W_SLAB):
            nc.sync.dma_start(
                w_sb[:, s:s + W_SLAB, :], wt_r[:, s:s + W_SLAB, :]
            )
        nc.sync.dma_start(bias_sb[:], brep)

        def eject(t, c, ps):
            o_sb = opool.tile([P, 512], mybir.dt.float32,
                              name="o_sb", tag="o_sb")[:, :CHUNKS[c]]
            nc.vector.tensor_add(o_sb[:], ps[:], bias_sb[:, coff[c]:coff[c + 1]])
            nc.sync.dma_start(y[t * P:(t + 1) * P, coff[c]:coff[c + 1]], o_sb[:])

        # t = 0 and 1 interleaved over ks: their combined compute (~37us)
        # covers the W-load tail so the PE never starves while W streams in.
        pss01 = [
            [
                pspool.tile([P, CHUNKS[c]], mybir.dt.float32,
                            name=f"ps{c}", tag=f"ps{c}")
                for c in range(len(CHUNKS))
            ]
            for _ in range(2)
        ]
        for ks in range(KS):
            for tt in range(2):
                x_sb = x0 if tt == 0 else x1
                for c in range(len(CHUNKS)):
                    nc.tensor.matmul(
                        pss01[tt][c][:],
                        x_sb[:, ks, :],
                        w_sb[:, ks, coff[c]:coff[c + 1]],
                        start=(ks == 0),
                        stop=(ks == KS - 1),
                    )
        for tt in range(2):
            for c in range(len(CHUNKS)):
                eject(tt, c, pss01[tt][c])

        for t in range(2, TT):
            x_sb = xpool.tile([P, KS, P], dt16, name="x_sb", tag="x_sb")
            nc.sync.dma_start(x_sb[:], xt_r[:, :, t * P:(t + 1) * P])

            pss = [
                pspool.tile([P, CHUNKS[c]], mybir.dt.float32,
                            name=f"ps{c}", tag=f"ps{c}")
                for c in range(len(CHUNKS))
            ]
            for ks in range(KS):
                for c in range(len(CHUNKS)):
                    nc.tensor.matmul(
                        pss[c][:],
                        x_sb[:, ks, :],
                        w_sb[:, ks, coff[c]:coff[c + 1]],
                        start=(ks == 0),
                        stop=(ks == KS - 1),
                    )
            for c in range(len(CHUNKS)):
                eject(t, c, pss[c])

    nc.compile()
    return nc


def _host_prep(x, w_packed, w_scale, w_bias, b):
    import ml_dtypes  # noqa: F401

    # Dequantize on host exactly as the reference does, then cast to fp16.
    shifts = np.array([12, 8, 4, 0], dtype=np.int32)
    nib = ((w_packed[..., None] >> shifts) & 15).astype(np.float32)
    n_rows, n_groups, n_ids = w_packed.shape
    W = nib.reshape(n_rows, n_groups, n_ids * 4) * w_scale + w_bias
    W = W.reshape(n_rows, n_groups * n_ids * 4)          # (out, in) fp32
    WT = np.ascontiguousarray(W.T.astype(np.float16))    # (in, out) fp16
    xT = np.ascontiguousarray(x.T.astype(np.float16))    # (in, tokens) fp16

    in_maps = []
    for i in range(N_CORES):
        sl = slice(i * SHARD, (i + 1) * SHARD)
        in_maps.append(
            {
                "xt": xT,
                "wt": np.ascontiguousarray(WT[:, sl]),
                "brep": np.ascontiguousarray(
                    np.broadcast_to(b[sl].astype(np.float32), (P, SHARD))
                ),
            }
        )
    return in_maps


def _run(x, w_packed, w_scale, w_bias, b, trace=False):
    from concourse.bass_utils import run_bass_kernel_spmd

    if "nc" not in _NC_CACHE:
        _NC_CACHE["nc"] = _build_nc()
    nc = _NC_CACHE["nc"]
    in_maps = _host_prep(x, w_packed, w_scale, w_bias, b)
    res = run_bass_kernel_spmd(nc, in_maps, list(range(N_CORES)), trace=trace)
    y = np.concatenate([res.results[i]["y"] for i in range(N_CORES)], axis=1)
    return np.ascontiguousarray(y.astype(np.float32)), res


def kernel(x, w_packed, w_scale, w_bias, b):
    x = np.asarray(x)
    w_packed = np.asarray(w_packed)
    w_scale = np.asarray(w_scale)
    w_bias = np.asarray(w_bias)
    b = np.asarray(b)
    y, _ = _run(x, w_packed, w_scale, w_bias, b, trace=False)
    return y


# revision 11
# speedup vs baseline: 1.0993x; 1.0006x over previous
"""GroupQuantLinear: y = x @ dequant(w).T + b on 8 NeuronCores.

Strategy (column-parallel / tensor-parallel over out_features):
  - Host: dequantize packed 4-bit weights -> W (out,in) fp32, cast to fp16,
    pre-transpose to WT (in,out); pre-transpose x -> xT (in,tokens) fp16.
  - Shard WT / bias along out_features across 8 cores (1376 each).
  - Each core: WT shard resident in SBUF (fp16, 11.3MB); stream 128-token
    tiles of xT; fp16 matmuls accumulate over K=4096 in fp32 PSUM
    (3 PSUM banks: 512/512/352 out-cols per token tile); add bias on
    copy-out; DMA fp32 output in natural (tokens, outs) layout.
  - W is loaded in ks-major slabs so the PE can start after ~2MB arrives.
  - Host: concatenate the 8 output shards along the out dim.
"""

import os
import sys
from contextlib import ExitStack

import numpy as np

sys.path.insert(0, "/opt/trn_rl_repo")

TOKENS = 8192
IN_F = 4096
OUT_F = 11008
N_CORES = 8
SHARD = OUT_F // N_CORES          # 1376
CHUNKS = (512, 512, 352)          # out-cols per PSUM bank, sum = SHARD
P = 128
KS = IN_F // P                    # 32
TT = TOKENS // P                  # 64
W_SLAB = 1                        # ks per W-load DMA slab (after the first 4)

_NC_CACHE = {}


def _build_nc():
    import concourse.bacc as bacc
    import concourse.mybir as mybir
    import concourse.tile as tile

    dt16 = mybir.dt.float16

    nc = bacc.Bacc(
        "TRN2",
        target_bir_lowering=False,
        debug=False,
        enable_asserts=False,
        num_devices=N_CORES,
    )
    xt = nc.dram_tensor("xt", (IN_F, TOKENS), dt16, kind="ExternalInput").ap()
    wt = nc.dram_tensor("wt", (IN_F, SHARD), dt16, kind="ExternalInput").ap()
    brep = nc.dram_tensor("brep", (P, SHARD), mybir.dt.float32, kind="ExternalInput").ap()
    y = nc.dram_tensor("y", (TOKENS, SHARD), mybir.dt.float32, kind="ExternalOutput").ap()

    coff = [0]
    for ch in CHUNKS:
        coff.append(coff[-1] + ch)

    with tile.TileContext(nc) as tc, ExitStack() as ctx:
        wpool = ctx.enter_context(tc.tile_pool(name="w", bufs=1))
        xpool = ctx.enter_context(tc.tile_pool(name="x", bufs=4))
        opool = ctx.enter_context(tc.tile_pool(name="o", bufs=6))
        pspool = ctx.enter_context(tc.tile_pool(name="ps", bufs=2, space="PSUM"))

        w_sb = wpool.tile([P, KS, SHARD], dt16, name="w_sb")
        bias_sb = wpool.tile([P, SHARD], mybir.dt.float32, name="bias_sb")

        xt_r = xt.rearrange("(ks p) m -> p ks m", p=P)
        wt_r = wt.rearrange("(ks p) n -> p ks n", p=P)

        # PE prewarm: dependency-free dummy matmuls on uninitialized SBUF.
        # They run during the initial DMA wait and lift HAM to 2.4GHz
        # before the first real matmul issues.
        warm_in = wpool.tile([P, P], dt16, name="warm_in")
        nc.any.memzero(warm_in[:])
        warm_ps = pspool.tile([P, P], mybir.dt.float32, name="warm_ps", tag="warm", bufs=1)
        for _ in range(60):
            nc.tensor.matmul(warm_ps[:], warm_in[:], warm_in[:], start=True, stop=True)

        # Early loads, balanced so x ks-slices land just ahead of their
        # consumption by the t0/t1-interleaved ks loop.
        x0 = xpool.tile([P, KS, P], dt16, name="x_sb", tag="x_sb")
        x1 = xpool.tile([P, KS, P], dt16, name="x_sb", tag="x_sb")
        nc.sync.dma_start(x0[:, 0:4, :], xt_r[:, 0:4, 0:P])
        nc.sync.dma_start(x1[:, 0:4, :], xt_r[:, 0:4, P:2 * P])
        q4 = SHARD // 4
        for q in range(4):
            nc.sync.dma_start(
                w_sb[:, 0:1, q * q4:(q + 1) * q4], wt_r[:, 0:1, q * q4:(q + 1) * q4]
            )
        nc.sync.dma_start(x0[:, 4:16, :], xt_r[:, 4:16, 0:P])
        nc.sync.dma_start(x1[:, 4:16, :], xt_r[:, 4:16, P:2 * P])
        half = SHARD // 2
        for s in range(1, 4):
            nc.sync.dma_start(w_sb[:, s:s + 1, :half], wt_r[:, s:s + 1, :half])
            nc.sync.dma_start(w_sb[:, s:s + 1, half:], wt_r[:, s:s + 1, half:])
        nc.sync.dma_start(x0[:, 16:KS, :], xt_r[:, 16:KS, 0:P])
        nc.sync.dma_start(x1[:, 16:KS, :], xt_r[:, 16:KS, P:2 * P])
        x2 = xpool.tile([P, KS, P], dt16, name="x_sb", tag="x_sb")
        nc.sync.dma_start(x2[:, 0:16, :], xt_r[:, 0:16, 2 * P:3 * P])
        for s in range(4, KS, W_SLAB):
            nc.sync.dma_start(
                w_sb[:, s:s + W_SLAB, :], wt_r[:, s:s + W_SLAB, :]
            )
            if s == 8:
                nc.sync.dma_start(x2[:, 16:KS, :], xt_r[:, 16:KS, 2 * P:3 * P])
        nc.sync.dma_start(bias_sb[:], brep)

        def eject(t, c, ps):
            o_sb = opool.tile([P, 512], mybir.dt.float32,
                              name="o_sb", tag="o_sb")[:, :CHUNKS[c]]
            nc.vector.tensor_add(o_sb[:], ps[:], bias_sb[:, coff[c]:coff[c + 1]])
            nc.sync.dma_start(y[t * P:(t + 1) * P, coff[c]:coff[c + 1]], o_sb[:])

        # t = 0 and 1 interleaved over ks: their combined compute (~37us)
        # covers the W-load tail so the PE never starves while W streams in.
        pss01 = [
            [
                pspool.tile([P, CHUNKS[c]], mybir.dt.float32,
                            name=f"ps{c}", tag=f"ps{c}")
                for c in range(len(CHUNKS))
            ]
            for _ in range(2)
        ]
        # t2's chunk 0 rides along in the spare PSUM bank, adding PE work
        # to the DMA-limited load window.
        ps2c0 = pspool.tile([P, CHUNKS[0]], mybir.dt.float32, name="ps2c0",
                            tag="ps2c0", bufs=1)
        for ks in range(KS):
            for tt in range(2):
                x_sb = x0 if tt == 0 else x1
                for c in range(len(CHUNKS)):
                    nc.tensor.matmul(
                        pss01[tt][c][:],
                        x_sb[:, ks, :],
                        w_sb[:, ks, coff[c]:coff[c + 1]],
                        start=(ks == 0),
                        stop=(ks == KS - 1),
                    )
            nc.tensor.matmul(
                ps2c0[:],
                x2[:, ks, :],
                w_sb[:, ks, coff[0]:coff[1]],
                start=(ks == 0),
                stop=(ks == KS - 1),
            )
        for tt in range(2):
            for c in range(len(CHUNKS)):
                eject(tt, c, pss01[tt][c])
        eject(2, 0, ps2c0)
        # t2's remaining chunks: second ks pass (W is fully resident now).
        pss2 = [
            pspool.tile([P, CHUNKS[c]], mybir.dt.float32, name=f"ps{c}", tag=f"ps{c}")
            for c in (1, 2)
        ]
        for ks in range(KS):
            for i, c in enumerate((1, 2)):
                nc.tensor.matmul(
                    pss2[i][:],
                    x2[:, ks, :],
                    w_sb[:, ks, coff[c]:coff[c + 1]],
                    start=(ks == 0),
                    stop=(ks == KS - 1),
                )
        for i, c in enumerate((1, 2)):
            eject(2, c, pss2[i])

        for t in range(3, TT):
            x_sb = xpool.tile([P, KS, P], dt16, name="x_sb", tag="x_sb")
            nc.sync.dma_start(x_sb[:], xt_r[:, :, t * P:(t + 1) * P])

            pss = [
                pspool.tile([P, CHUNKS[c]], mybir.dt.float32,
                            name=f"ps{c}", tag=f"ps{c}")
                for c in range(len(CHUNKS))
            ]
            for ks in range(KS):
                for c in range(len(CHUNKS)):
                    nc.tensor.matmul(
                        pss[c][:],
                        x_sb[:, ks, :],
                        w_sb[:, ks, coff[c]:coff[c + 1]],
                        start=(ks == 0),
                        stop=(ks == KS - 1),
                    )
            for c in range(len(CHUNKS)):
                eject(t, c, pss[c])

    nc.compile()
    return nc


def _host_prep(x, w_packed, w_scale, w_bias, b):
    import ml_dtypes  # noqa: F401

    # Dequantize on host exactly as the reference does, then cast to fp16.
    shifts = np.array([12, 8, 4, 0], dtype=np.int32)
    nib = ((w_packed[..., None] >> shifts) & 15).astype(np.float32)
    n_rows, n_groups, n_ids = w_packed.shape
    W = nib.reshape(n_rows, n_groups, n_ids * 4) * w_scale + w_bias
    W = W.reshape(n_rows, n_groups * n_ids * 4)          # (out, in) fp32
    WT = np.ascontiguousarray(W.T.astype(np.float16))    # (in, out) fp16
    xT = np.ascontiguousarray(x.T.astype(np.float16))    # (in, tokens) fp16

    in_maps = []
    for i in range(N_CORES):
        sl = slice(i * SHARD, (i + 1) * SHARD)
        in_maps.append(
            {
                "xt": xT,
                "wt": np.ascontiguousarray(WT[:, sl]),
                "brep": np.ascontiguousarray(
                    np.broadcast_to(b[sl].astype(np.float32), (P, SHARD))
                ),
            }
        )
    return in_maps


def _run(x, w_packed, w_scale, w_bias, b, trace=False):
    from concourse.bass_utils import run_bass_kernel_spmd

    if "nc" not in _NC_CACHE:
        _NC_CACHE["nc"] = _build_nc()
    nc = _NC_CACHE["nc"]
    in_maps = _host_prep(x, w_packed, w_scale, w_bias, b)
    res = run_bass_kernel_spmd(nc, in_maps, list(range(N_CORES)), trace=trace)
    y = np.concatenate([res.results[i]["y"] for i in range(N_CORES)], axis=1)
    return np.ascontiguousarray(y.astype(np.float32)), res


def kernel(x, w_packed, w_scale, w_bias, b):
    x = np.asarray(x)
    w_packed = np.asarray(w_packed)
    w_scale = np.asarray(w_scale)
    w_bias = np.asarray(w_bias)
    b = np.asarray(b)
    y, _ = _run(x, w_packed, w_scale, w_bias, b, trace=False)
    return y
